# revision 27
# baseline (speedup 1.0000x reference)
# MoE (top-2 of 8 experts) kernel for 8 Trainium2 NeuronCores.
#
# Strategy: expert-parallel sparse routing. The reference computes every
# expert densely, but the output only depends on each token's top-2
# experts. Host computes the (tiny) gating network and per-expert token
# lists; core e runs expert e's FFN (x@W1+b1 -> LayerNorm -> erf-GELU ->
# @W2+b2) on just its routed tokens; host applies the gate weights in
# the combine. Matmuls run in float32r (TF32) at full PE rate, tokens on
# the moving (free) dimension for both matmuls so no on-device
# transposes are needed. LayerNorm reductions over H (the partition dim)
# are done with ones-vector matmuls on the PE; per-token stats are
# broadcast back across partitions with K=1 matmuls.

import tempfile

import numpy as np

import concourse.bacc as bacc
import concourse.mybir as mybir
import concourse.tile as tile
from concourse._compat import axon_active
from concourse.bass_utils import run_bass_kernel_spmd

P = 128
D, H, E, TOPK = 1024, 2048, 8, 2
DS, NJ, NK = D // P, H // P, H // P
LN_EPS = 1e-5
TT = 512          # main token tile (fp32 moving-operand max)
TT_MIN = 256      # capacity granularity; tail tiles use native-shape SBUF tiles
NJC = 8           # W2 js-slices cached in SBUF; NJ-NJC streamed per k-tile

_kernel_cache: dict[int, object] = {}


def _t_tiles(C):
    tiles, t0 = [], 0
    while t0 < C:
        tt = TT if C - t0 >= TT else TT_MIN
        tiles.append((t0, tt))
        t0 += tt
    # Put the smaller tail tile second: its DMA-heavy mm1 overlaps the
    # first 512-tile's mm2, and the final (non-overlapped) mm2 is a full
    # 512 tile with the best compute:DMA ratio.
    if len(tiles) > 1 and tiles[-1][1] != TT:
        tiles = [tiles[0], tiles[-1]] + tiles[1:-1]
    return tiles


def _build(C: int):
    f32, f32r = mybir.dt.float32, mybir.dt.float32r
    nc = bacc.Bacc("TRN2", target_bir_lowering=False, debug=False, num_devices=8)
    xT = nc.dram_tensor("xT", [P, DS, C], f32r, kind="ExternalInput").ap()
    W1 = nc.dram_tensor("W1", [NJ, P, DS, P], f32r, kind="ExternalInput").ap()
    W2 = nc.dram_tensor("W2", [P, NJC, H], f32r, kind="ExternalInput").ap()
    W2s = nc.dram_tensor("W2s", [NK, P, NJ - NJC, P], f32r, kind="ExternalInput").ap()
    b1 = nc.dram_tensor("b1", [P, NJ], f32, kind="ExternalInput").ap()
    lg = nc.dram_tensor("lg", [P, NJ], f32, kind="ExternalInput").ap()
    lb = nc.dram_tensor("lb", [P, NJ], f32, kind="ExternalInput").ap()
    b2 = nc.dram_tensor("b2", [P, NK], f32, kind="ExternalInput").ap()
    outT = nc.dram_tensor("outT", [NK, P, C], f32, kind="ExternalOutput").ap()

    Gelu = mybir.ActivationFunctionType.Gelu
    Ident = mybir.ActivationFunctionType.Identity
    Sqrt = mybir.ActivationFunctionType.Sqrt

    with tile.TileContext(nc) as tc:
        with (
            tc.tile_pool(name="const", bufs=1) as constp,
            tc.tile_pool(name="w2p", bufs=1) as w2p,
            tc.tile_pool(name="w1p", bufs=2) as w1p,
            tc.tile_pool(name="w2sp", bufs=2) as w2sp,
            tc.tile_pool(name="xp", bufs=1) as xp,
            tc.tile_pool(name="hp", bufs=2) as hp,
            tc.tile_pool(name="sqp", bufs=2) as sqp,
            tc.tile_pool(name="op", bufs=2) as op,
            tc.tile_pool(name="statp", bufs=1) as statp,
            tc.tile_pool(name="ps_mm", bufs=3, space="PSUM") as ps_mm,
            tc.tile_pool(name="ps_acc", bufs=1, space="PSUM") as ps_acc,
            tc.tile_pool(name="ps_bc", bufs=1, space="PSUM") as ps_bc,
        ):
            b1s = constp.tile([P, NJ], f32)
            nc.sync.dma_start(b1s[:], b1[:])
            lgs = constp.tile([P, NJ], f32)
            nc.sync.dma_start(lgs[:], lg[:])
            lbs = constp.tile([P, NJ], f32)
            nc.sync.dma_start(lbs[:], lb[:])
            b2s = constp.tile([P, NK], f32)
            nc.sync.dma_start(b2s[:], b2[:])
            ones_f = constp.tile([P, 1], f32)
            nc.any.memset(ones_f[:], 1.0)
            ones_c = constp.tile([P, 1], f32r)  # lhsT for partition-sum matmuls
            nc.vector.tensor_copy(ones_c[:], ones_f[:])
            oner_f = constp.tile([1, P], f32)
            nc.any.memset(oner_f[:], 1.0)
            oner_c = constp.tile([1, P], f32r)  # lhsT for partition-broadcast matmuls
            nc.vector.tensor_copy(oner_c[:], oner_f[:])
            eps_t = constp.tile([1, 1], f32)
            nc.any.memset(eps_t[:], LN_EPS)

            # Cache js slices [0, NJC) of W2 in SBUF (f32r); stream the rest
            # per k-tile (they don't fit alongside both t-tile shapes). The
            # DMAs are emitted after the first tile's xt/W1 loads (below) so
            # they don't delay the first matmuls; first use is ~90us in.
            w2sb = w2p.tile([P, NJC, H], f32r)

            def emit_w2c_chunk(c):
                if c < NJC:
                    nc.sync.dma_start(
                        w2sb[:, c : c + 1, :], W2[:, c : c + 1, :]
                    )

            def emit_mm2(h, t0, tt):
                for k in range(NK):
                    w2t = w2sp.tile([P, NJ - NJC, P], f32r, tag="w2s", name="w2t")
                    nc.sync.dma_start(w2t[:], W2s[k])
                    pm = ps_mm.tile([P, TT], f32, tag="mm", name="mm")[:, :tt]
                    for js in range(NJ):
                        nc.tensor.matmul(
                            pm[:],
                            w2sb[:, js, k * P : (k + 1) * P]
                            if js < NJC
                            else w2t[:, js - NJC, :],
                            h[:, js, :],
                            start=(js == 0),
                            stop=(js == NJ - 1),
                        )
                    ot = op.tile(
                        [P, tt], f32, tag=f"out{tt}", name="out",
                        bufs=(2 if tt == TT else 1),
                    )
                    nc.scalar.activation(ot[:], pm[:], Ident, bias=b2s[:, k : k + 1])
                    nc.sync.dma_start(outT[k, :, t0 : t0 + tt], ot[:])

            # Software-pipelined emission: tile i's mm2 is emitted after tile
            # i+1's mm1, so the PE runs mm2(i) while ACT/DVE do tile i+1's
            # LayerNorm stats, and runs mm1(i+1) while normalize/gelu(i+1)
            # complete. h is double-buffered to allow it.
            prev = None
            first = True
            for t0, tt in _t_tiles(C):
                xt = xp.tile([P, DS, tt], f32r, tag=f"xt{tt}", name="xt")
                nc.sync.dma_start(xt[:], xT[:, :, t0 : t0 + tt])
                h = hp.tile(
                    [P, NJ, tt], f32r, tag=f"h{tt}", name="h",
                    bufs=(2 if tt == TT else 1),
                )
                s_ps = ps_acc.tile([1, TT], f32, tag="sacc", name="sacc")[:, :tt]
                q_ps = ps_acc.tile([1, TT], f32, tag="qacc", name="qacc")[:, :tt]

                # ---- mm1; S/Q ones-matmuls deferred by one j so the PE
                # never waits on the ACT evict / DVE square chain ----
                def emit_snq(j, sq):
                    nc.tensor.matmul(
                        s_ps[:], ones_c[:], h[:, j, :],
                        start=(j == 0), stop=(j == NJ - 1),
                    )
                    nc.tensor.matmul(
                        q_ps[:], ones_c[:], sq[:],
                        start=(j == 0), stop=(j == NJ - 1),
                    )

                pend = None
                for j in range(NJ):
                    w1t = w1p.tile([P, DS, P], f32r, tag="w1")
                    nc.sync.dma_start(w1t[:], W1[j])
                    if first:
                        emit_w2c_chunk(j)
                        emit_w2c_chunk(j + NJ)
                    pm = ps_mm.tile([P, TT], f32, tag="mm", name="mm")[:, :tt]
                    for ds in range(DS):
                        nc.tensor.matmul(
                            pm[:],
                            w1t[:, ds, :],
                            xt[:, ds, :],
                            start=(ds == 0),
                            stop=(ds == DS - 1),
                        )
                    # evict psum -> h (f32r) with per-partition bias b1[j]
                    nc.scalar.activation(
                        h[:, j, :], pm[:], Ident, bias=b1s[:, j : j + 1]
                    )
                    sq = sqp.tile([P, tt], f32r, tag=f"sq{tt}", name="sq")
                    nc.vector.tensor_mul(sq[:], h[:, j, :], h[:, j, :])
                    if pend is not None:
                        emit_snq(*pend)
                    pend = (j, sq)
                emit_snq(*pend)
                first = False

                # previous tile's mm2 fills the PE while this tile's stats run
                if prev is not None:
                    emit_mm2(*prev)

                # ---- LN stats -> per-token scale A=rstd and offset B=mu*rstd ----
                mu = statp.tile([1, TT], f32, tag="mu", name="mu")[:, :tt]
                nc.vector.tensor_scalar_mul(mu[:], s_ps[:], 1.0 / H)
                tmp = statp.tile([1, TT], f32, tag="tmp", name="tmp")[:, :tt]
                nc.vector.tensor_scalar_mul(tmp[:], q_ps[:], 1.0 / H)
                tmp2 = statp.tile([1, TT], f32, tag="tmp2", name="tmp2")[:, :tt]
                nc.vector.tensor_mul(tmp2[:], mu[:], mu[:])
                nc.vector.tensor_sub(tmp[:], tmp[:], tmp2[:])          # var
                nc.scalar.activation(tmp2[:], tmp[:], Sqrt, bias=eps_t[:])  # std
                nc.vector.reciprocal(tmp[:], tmp2[:])                  # rstd
                a_row = statp.tile([1, TT], f32r, tag="a_row", name="a_row")[:, :tt]
                nc.vector.tensor_copy(a_row[:], tmp[:])
                b_row = statp.tile([1, TT], f32r, tag="b_row", name="b_row")[:, :tt]
                nc.vector.tensor_mul(b_row[:], mu[:], tmp[:])
                # broadcast across partitions via K=1 matmuls
                a_bc = ps_bc.tile([P, TT], f32, tag="a_bc", name="a_bc")[:, :tt]
                nc.tensor.matmul(a_bc[:], oner_c[:], a_row[:], start=True, stop=True)
                b_bc = ps_bc.tile([P, TT], f32, tag="b_bc", name="b_bc")[:, :tt]
                nc.tensor.matmul(b_bc[:], oner_c[:], b_row[:], start=True, stop=True)

                # ---- normalize + affine + GELU (in place on h) ----
                for j in range(NJ):
                    hj = h[:, j, :]
                    nc.vector.tensor_mul(hj, hj, a_bc[:])
                    nc.vector.tensor_sub(hj, hj, b_bc[:])
                    nc.scalar.activation(
                        hj, hj, Gelu, bias=lbs[:, j : j + 1], scale=lgs[:, j : j + 1]
                    )
                prev = (h, t0, tt)

            emit_mm2(*prev)

    nc.compile()
    return nc


def _route(x64, Wg64, bg64):
    """Host gating: returns per-token top-2 expert ids and renormalized weights."""
    logits = x64 @ Wg64 + bg64                      # [N, E] fp64
    order = np.argsort(-logits, axis=1, kind="stable")[:, :TOPK]
    l0 = np.take_along_axis(logits, order, axis=1)  # [N, 2] descending
    # pair softmax == softmax-then-renormalize over the top-2
    w0 = 1.0 / (1.0 + np.exp(l0[:, 1] - l0[:, 0]))
    w = np.stack([w0, 1.0 - w0], axis=1)
    return order, w


def kernel(x, W1, b1, ln_g, ln_b, W2, b2, Wg, bg):
    x = np.ascontiguousarray(np.asarray(x, dtype=np.float32))
    W1 = np.asarray(W1, dtype=np.float32)
    b1 = np.asarray(b1, dtype=np.float32)
    ln_g = np.asarray(ln_g, dtype=np.float32)
    ln_b = np.asarray(ln_b, dtype=np.float32)
    W2 = np.asarray(W2, dtype=np.float32)
    b2 = np.asarray(b2, dtype=np.float32)
    Wg = np.asarray(Wg, dtype=np.float32)
    bg = np.asarray(bg, dtype=np.float32)
    N = x.shape[0]

    order, w = _route(x.astype(np.float64), Wg.astype(np.float64), bg.astype(np.float64))

    # Per-expert token lists, padded to a common capacity C (multiple of TT_MIN).
    tok_idx, tok_w = [], []
    for e in range(E):
        sel = np.nonzero((order[:, 0] == e) | (order[:, 1] == e))[0]
        we = np.where(order[sel, 0] == e, w[sel, 0], w[sel, 1]).astype(np.float32)
        tok_idx.append(sel)
        tok_w.append(we)
    C = max(TT_MIN, int(-(-max(len(s) for s in tok_idx) // TT_MIN)) * TT_MIN)

    if C not in _kernel_cache:
        _kernel_cache[C] = _build(C)
    nc = _kernel_cache[C]

    in_maps = []
    for e in range(E):
        idx = np.zeros(C, dtype=np.int64)
        idx[: len(tok_idx[e])] = tok_idx[e]
        xg = x[idx]                                   # [C, D]
        xT_dev = np.ascontiguousarray(xg.reshape(C, DS, P).transpose(2, 1, 0))
        W1_dev = np.ascontiguousarray(
            W1[e].reshape(DS, P, NJ, P).transpose(2, 1, 0, 3)
        )
        w2r = W2[e].reshape(NJ, P, H)
        W2_dev = np.ascontiguousarray(w2r[:NJC].transpose(1, 0, 2))
        W2s_dev = np.ascontiguousarray(
            W2[e][NJC * P :, :].reshape(NJ - NJC, P, NK, P).transpose(2, 1, 0, 3)
        )
        in_maps.append(
            {
                "xT": xT_dev,
                "W1": W1_dev,
                "W2": W2_dev,
                "W2s": W2s_dev,
                "b1": np.ascontiguousarray(b1[e].reshape(NJ, P).T),
                "lg": np.ascontiguousarray(ln_g[e].reshape(NJ, P).T),
                "lb": np.ascontiguousarray(ln_b[e].reshape(NJ, P).T),
                "b2": np.ascontiguousarray(b2[e].reshape(NK, P).T),
            }
        )

    results = _run(C, nc, in_maps)

    y = np.zeros((N, H), dtype=np.float32)
    for e in range(E):
        cnt = len(tok_idx[e])
        eoT = results[e]["outT"].reshape(H, C)
        y[tok_idx[e]] += tok_w[e][:, None] * eoT[:, :cnt].T
    return y


_neff_cache: dict[int, str] = {}


def _run(C, nc, in_maps):
    if axon_active():
        # PJRT path; NEFF compile is cached by libneuronxla.
        return run_bass_kernel_spmd(nc, in_maps, core_ids=list(range(E))).results
    # Native path: compile once per capacity, then execute the cached NEFF.
    from concourse.bass_utils import compile_bass_kernel, run_neff

    if C not in _neff_cache:
        _neff_cache[C] = compile_bass_kernel(nc, tempfile.mkdtemp())
    out_maps = [
        {"outT": np.zeros((NK, P, C), dtype=np.float32)} for _ in range(E)
    ]
    in_maps = [m.copy() for m in in_maps]
    if nc.partition_id_tensor:
        for core_id, m in enumerate(in_maps):
            m[nc.partition_id_tensor.name] = np.array([[core_id]], dtype=np.uint32)
    return run_neff(
        _neff_cache[C],
        in_maps,
        out_maps,
        core_ids=list(range(E)),
        has_collectives=False,
    )
